# revision 38
# baseline (speedup 1.0000x reference)
"""Fused multi-head attention layer for Trainium2, 8-core data-parallel.

Problem: x[8,1024,768] -> qkv proj (w_qkv[2304,768]) -> 12-head attention
(head_dim 64, key-padding mask) -> out proj (w_proj[768,768] + b_proj).

Strategy (v2):
  * Data parallel over batch: core b handles x[b] end to end. No collectives.
  * All matmul operands are bf16 (host-converted); PSUM accumulation is fp32,
    so the output error stays ~1e-3 relative. Halves DMA traffic and enables
    fast weight loads on the PE.
  * Host pre-transposes x / w_qkv / w_proj so every device matmul is
    native-layout (contraction dim on partitions). w_qkv is additionally laid
    out e-major-chunked so every weight DMA is fully contiguous.
  * QK^T is computed as qkvT[e,l] (e on partitions) so per-head Q^T/K^T
    [64,1024] slices are direct matmul operands; scores are computed
    TRANSPOSED: S.T[m,l] = K @ Q.T. The two heads of a pair live on
    partitions 0:64 / 64:128, so their K=64 score matmuls are packed into the
    PE array as 4 concurrent tile_position sub-tiles (2 row x 2 col groups)
    -- full-array utilization despite the 64-deep contraction.
  * Softmax runs without max-subtraction (scores are O(1) by construction);
    exp is a single scalar-engine activation (key-padding mask as additive
    per-partition bias, 1/sqrt(hd) folded into the activation scale), output
    directly in bf16 as the AV rhs. No P transpose anywhere.
  * Phase B is software-pipelined: the S burst for step j+1 is issued to the
    PE before the AV matmuls of step j, so the PE never head-of-line blocks
    on the scalar engine's exp. Phase B runs at the ACT engine's exp rate.
  * The softmax denominator comes free from a ones column appended to V
    (row 64 of the AV accumulator). Normalization (reciprocal + partition
    broadcast + multiply) runs off the critical path on DVE/GPSIMD.
  * PSUM->SBUF evacuations run on DVE (phases A/B) and ACT (phase C), keeping
    the scalar engine free for the exp stream in phase B.
"""

import os
import sys

import numpy as np

sys.path.insert(0, "/opt/trn_rl_repo")

B, L, D, H, HD = 8, 1024, 768, 12, 64
E = 3 * D
SCALE = HD ** -0.5
P = 128
KC = D // P          # 6 contraction chunks of 128 over d
LT = L // P          # 8 l/m partition tiles
NP = H // 2          # 6 head pairs
NCORES = 8
NEG = -30000.0       # mask bias; exp(NEG + s) == 0 in fp32
# w_qkv e-major DMA chunk boundaries (Q/K heads in chunks 0-2, V in 3-4)
ECHUNKS = [(0, 512), (512, 512), (1024, 512), (1536, 512), (2048, 256)]

_cached = {}


def _build_program(reps=1, phases='ABC', loop_n=0):
    import concourse.tile as tile
    from concourse import bacc, mybir

    f32 = mybir.dt.float32
    bf16 = mybir.dt.bfloat16
    AF = mybir.ActivationFunctionType

    nc = bacc.Bacc(trn_type="TRN2", target_bir_lowering=False, debug=False)

    # host pre-swizzled layouts: partition-major, contiguous per partition
    xT_d = nc.declare_dram_parameter("xT", [P, KC * L], bf16, isOutput=False)
    w1e_d = nc.declare_dram_parameter("w1e", [P, KC * E], bf16, isOutput=False)
    w2T_d = nc.declare_dram_parameter("w2T", [P, KC * D], bf16, isOutput=False)
    b2bc_d = nc.declare_dram_parameter("b2bc", [P, D], f32, isOutput=False)
    mbias_d = nc.declare_dram_parameter("mbias", [P, LT], f32, isOutput=False)
    ones_d = nc.declare_dram_parameter("ones", [P, H + 1], bf16, isOutput=False)
    out_d = nc.declare_dram_parameter("out", [P, LT * D], f32, isOutput=True)

    with tile.TileContext(nc) as tc:
      from contextlib import ExitStack, nullcontext

      with tc.For_i(0, loop_n, 1) if loop_n else nullcontext():
       for _rep in range(reps):
        with ExitStack() as ctx:
            if "Z" in phases:
                continue
            persist = ctx.enter_context(tc.tile_pool(name="persist", bufs=1))
            # qkvT for Q and K: e-tiles 0..5 = Q heads (2 per tile), 6..11 = K
            qkT_sb = persist.tile([P, 2 * KC, L], bf16)
            # partition-swapped duplicate (head A's Q/K also on partitions
            # 64:128 and vice versa) so each head's two c-half score matmuls
            # can occupy disjoint PE row groups and run concurrently
            qkT2_sb = persist.tile([P, 2 * KC, L], bf16)
            # V with a ones column per head: [l-tile, head, 65]
            V_sb = persist.tile([P, LT, H * (HD + 1)], bf16)
            V_v = V_sb[:].rearrange("p l (h c) -> p l h c", c=HD + 1)
            OT_sb = persist.tile([P, KC, L], bf16)      # O.T, heads stacked
            bias_sb = persist.tile([P, LT], f32)        # mask bias per key pos
            w2Tb_sb = persist.tile([P, KC, D], bf16)
            b2bc_sb = persist.tile([P, D], f32)
            out_sb = persist.tile([P, LT, D], f32)

            for j in range(LT):
                nc.scalar.dma_start(
                    out=V_v[:, j, :, HD], in_=ones_d[:, 0:H]
                )
            nc.scalar.dma_start(out=bias_sb[:], in_=mbias_d.ap())

            # ---------------- Phase A: QKV projection ----------------
            # xT / w1T stay resident through phase B: only the Q/K e-tiles
            # of the first head pair (et 0 and 6) and all of V are computed
            # up front; the remaining e-tiles are interleaved into phase B,
            # riding in the PE slack under the ACT-bound exp stream.
            pA = ctx.enter_context(tc.tile_pool(name="phA", bufs=1))
            xT_sb = pA.tile([P, KC, L], bf16)
            w1T_sb = pA.tile([P, KC, E], bf16)
            xT_r = xT_d.ap().rearrange("p (k l) -> p k l", l=L)
            # chunked loads so the first matmuls start early; w1e is
            # e-major on the host so every chunk is contiguous
            for k in range(KC):
                nc.sync.dma_start(out=xT_sb[:, k, :], in_=xT_r[:, k, :])
            off = 0
            for e0, ew in ECHUNKS:
                # separate DGE queue from the xT loads so both stream at once
                nc.gpsimd.dma_start(
                    out=w1T_sb[:, :, e0 : e0 + ew],
                    in_=w1e_d[:, off : off + KC * ew].rearrange(
                        "p (k e) -> p k e", e=ew
                    ),
                )
                off += KC * ew
            if "D" in phases:
                continue

            def qk_tile(psp, et, evac_dve):
                # qkT[e,l] = w1.T.T @ xT for e-tile et; k-outer / c-inner so
                # the two per-bank accumulation chains interleave, hiding
                # chain-start bubbles
                ps = psp.tile([P, L], f32, tag="qk")
                for k in range(KC):
                    for c in range(2):
                        nc.tensor.matmul(
                            ps[:, c * 512 : (c + 1) * 512],
                            lhsT=w1T_sb[:, k, et * P : (et + 1) * P],
                            rhs=xT_sb[:, k, c * 512 : (c + 1) * 512],
                            start=(k == 0),
                            stop=(k == KC - 1),
                        )
                if evac_dve:
                    nc.vector.tensor_copy(qkT_sb[:, et, :], ps[:])
                else:
                    nc.scalar.copy(qkT_sb[:, et, :], ps[:])
                # partition-swapped duplicate via DMA (engines cannot move
                # data across partitions); Pool DGE queue, off critical path
                nc.gpsimd.dma_start(
                    out=qkT2_sb[0:64, et, :], in_=qkT_sb[64:128, et, :]
                )
                nc.gpsimd.dma_start(
                    out=qkT2_sb[64:128, et, :], in_=qkT_sb[0:64, et, :]
                )

            with tc.tile_pool(name="psQK", bufs=3, space="PSUM") as psQK:
                for et in range(2 * KC):
                    qk_tile(psQK, et, et % 2 == 1)

            # V[l, dv] = x @ w1_v.T  (dv in [1536, 2304))
            with tc.tile_pool(name="psV", bufs=3, space="PSUM") as psV:
                for i in range(LT):
                    ps = psV.tile([P, D], f32, tag="v")
                    for k in range(KC):
                        for c0, cw in ((0, 512), (512, 256)):
                            nc.tensor.matmul(
                                ps[:, c0 : c0 + cw],
                                lhsT=xT_sb[:, k, i * P : (i + 1) * P],
                                rhs=w1T_sb[:, k, 2 * D + c0 : 2 * D + c0 + cw],
                                start=(k == 0),
                                stop=(k == KC - 1),
                            )
                    for c in range(2):
                        dst = V_v[:, i, 6 * c : 6 * (c + 1), 0:HD]
                        src = ps[:, c * 384 : (c + 1) * 384].rearrange(
                            "p (h q) -> p h q", q=HD
                        )
                        if (i + c) % 2 == 0:
                            nc.scalar.copy(dst, src)
                        else:
                            nc.vector.tensor_copy(dst, src)

            if "B" not in phases:
                continue
            # -------- Phase B: attention (+ prefetch of phase C inputs) -----
            nc.scalar.dma_start(
                out=w2Tb_sb[:, :, :],
                in_=w2T_d.ap().rearrange("p (k f) -> p k f", f=D),
            )
            nc.scalar.dma_start(out=b2bc_sb[:], in_=b2bc_d.ap())

            with tc.tile_pool(name="pt", bufs=3) as ptp, tc.tile_pool(
                name="norm", bufs=1
            ) as pn, tc.tile_pool(name="psS", bufs=2, space="PSUM") as psS, tc.tile_pool(
                name="psO", bufs=1, space="PSUM"
            ) as psO:
                # Pair structure: per j the ACT engine runs two exps
                # (~2.3us) -- a two-instruction backlog that rides through
                # PE jitter -- while the PE runs one head's S matmuls (the
                # two c-halves concurrently on disjoint row groups via the
                # partition-swapped qkT duplicate) plus its AV (~1.7us).
                for t in range(NP):
                    if "X" not in phases:
                        oA = psO.tile([P, L], f32, tag="oA")
                        oB = psO.tile([P, L], f32, tag="oB")
                        otiles = (oA, oB)

                    def s_burst(j):
                        # per head: c0 from the native partitions, c1 from
                        # the swapped duplicate -> disjoint PE row groups,
                        # both gated on the same exp -> issued adjacently
                        sA = psS.tile([P, L], f32, tag="s")
                        sB = psS.tile([P, L], f32, tag="s")
                        for pst, kb in ((sA, 0), (sB, 64)):
                            nc.tensor.matmul(
                                pst[:, 0:512],
                                lhsT=qkT_sb[
                                    kb : kb + 64, KC + t, j * P : (j + 1) * P
                                ],
                                rhs=qkT_sb[kb : kb + 64, t, 0:512],
                                start=True,
                                stop=True,
                            )
                            kb2 = 64 - kb
                            nc.tensor.matmul(
                                pst[:, 512:1024],
                                lhsT=qkT2_sb[
                                    kb2 : kb2 + 64, KC + t, j * P : (j + 1) * P
                                ],
                                rhs=qkT2_sb[kb2 : kb2 + 64, t, 512:1024],
                                start=True,
                                stop=True,
                            )
                        return sA, sB

                    stiles = s_burst(0)
                    for j in range(LT):
                        sA, sB = stiles
                        pts = []
                        for hh, spst in ((0, sA), (1, sB)):
                            pt_t = ptp.tile([P, L], bf16, tag=f"pt{hh}")
                            if "U" in phases:
                                nc.scalar.activation(pt_t[:], spst[:], AF.Exp)
                            else:
                                nc.scalar.activation(
                                    pt_t[:],
                                    spst[:],
                                    AF.Exp,
                                    bias=bias_sb[:, j : j + 1],
                                    scale=SCALE,
                                )
                            pts.append(pt_t)
                        if j + 1 < LT:
                            stiles = s_burst(j + 1)
                        if "X" in phases:
                            continue
                        for hh in range(2):
                            h = 2 * t + hh
                            for c in range(2):
                                nc.tensor.matmul(
                                    otiles[hh][0:65, c * 512 : (c + 1) * 512],
                                    lhsT=V_v[:, j, h, :],
                                    rhs=pts[hh][:, c * 512 : (c + 1) * 512],
                                    start=(j == 0),
                                    stop=(j == LT - 1),
                                )
                    if "X" in phases:
                        continue
                    # stage O' to SBUF fast (frees the PSUM accumulators),
                    # then normalize off the critical path
                    osA = pn.tile([65, L], f32, tag="osA")
                    osB = pn.tile([65, L], f32, tag="osB")
                    nc.vector.tensor_copy(osA[:], oA[0:65, :])
                    nc.vector.tensor_copy(osB[:], oB[0:65, :])
                    if "N" in phases:
                        continue
                    # move denominator rows to physical partition 0
                    # (partition_broadcast only reads partition 0 on HW)
                    den0 = pn.tile([1, 2, L], f32, tag="den0")
                    nc.sync.dma_start(out=den0[0:1, 0, :], in_=osA[64:65, :])
                    nc.sync.dma_start(out=den0[0:1, 1, :], in_=osB[64:65, :])
                    denr = pn.tile([1, 2, L], f32, tag="denr")
                    nc.vector.reciprocal_approx_fast(
                        denr[0:1, :, :], den0[0:1, :, :]
                    )
                    rep = pn.tile([64, 2, L], f32, tag="rep")
                    nc.gpsimd.partition_broadcast(
                        rep[0:64, 0, :], denr[0:1, 0, :], channels=64
                    )
                    nc.gpsimd.partition_broadcast(
                        rep[0:64, 1, :], denr[0:1, 1, :], channels=64
                    )
                    btmp = pn.tile([64, L], bf16, tag="btmp")
                    nc.vector.tensor_mul(
                        OT_sb[0:64, t, :], osA[0:64, :], rep[0:64, 0, :]
                    )
                    nc.vector.tensor_mul(
                        btmp[0:64, :], osB[0:64, :], rep[0:64, 1, :]
                    )
                    nc.sync.dma_start(out=OT_sb[64:128, t, :], in_=btmp[0:64, :])

            if "C" not in phases:
                continue
            # ---------------- Phase C: output projection ----------------
            with tc.tile_pool(name="psC", bufs=3, space="PSUM") as psC:
                out_r = out_d.ap().rearrange("p (i f) -> p i f", f=D)
                for i in range(LT):
                    ps = psC.tile([P, D], f32, tag="prj")
                    for k in range(KC):
                        for c0, cw in ((0, 512), (512, 256)):
                            nc.tensor.matmul(
                                ps[:, c0 : c0 + cw],
                                lhsT=OT_sb[:, k, i * P : (i + 1) * P],
                                rhs=w2Tb_sb[:, k, c0 : c0 + cw],
                                start=(k == 0),
                                stop=(k == KC - 1),
                            )
                    # bias add fused into the PSUM evacuation on DVE
                    nc.vector.tensor_add(out_sb[:, i, :], ps[:], b2bc_sb[:])
                    nc.sync.dma_start(out=out_r[:, i, :], in_=out_sb[:, i, :])

    nc.compile()
    return nc


def _get_program(reps=1, phases="ABC", loop_n=0):
    key = f"nc{reps}{phases}L{loop_n}"
    if key not in _cached:
        _cached[key] = _build_program(reps, phases, loop_n)
    return _cached[key]


def _prep_inputs(x, attn_mask, w_qkv, w_proj, b_proj):
    import ml_dtypes

    BF16 = np.dtype(ml_dtypes.bfloat16)
    x = np.asarray(x, dtype=np.float32)
    attn_mask = np.asarray(attn_mask)
    w1T = np.ascontiguousarray(np.asarray(w_qkv, np.float32).T)        # [768, 2304]
    w2T = np.ascontiguousarray(np.asarray(w_proj, np.float32).T)       # [768, 768]

    def swz(a, inner):
        # [KC*P, inner] -> [P, KC*inner], partition-major contiguous
        return np.ascontiguousarray(
            a.reshape(KC, P, inner).transpose(1, 0, 2).reshape(P, KC * inner)
        )

    w1k = swz(w1T, E).reshape(P, KC, E)
    # e-major chunking so each weight DMA reads a contiguous range
    w1e = np.concatenate(
        [w1k[:, :, e0 : e0 + ew].reshape(P, KC * ew) for e0, ew in ECHUNKS],
        axis=1,
    ).astype(BF16)
    w2Ts = swz(w2T, D).astype(BF16)
    b2bc = np.ascontiguousarray(
        np.broadcast_to(np.asarray(b_proj, np.float32)[None, :], (P, D))
    )
    ones = np.ones((P, H + 1), BF16)
    in_maps = []
    for b in range(B):
        xT = swz(np.ascontiguousarray(x[b].T), L).astype(BF16)          # [128, 6144]
        mb = NEG * (1 - attn_mask[b].astype(np.float32))                # [1024]
        mbs = np.ascontiguousarray(mb.reshape(LT, P).T.astype(np.float32))
        in_maps.append(
            {
                "xT": xT,
                "w1e": w1e,
                "w2T": w2Ts,
                "b2bc": b2bc,
                "mbias": mbs,
                "ones": ones,
            }
        )
    return in_maps


def run(x, attn_mask, w_qkv, w_proj, b_proj, trace=False, **spmd_kwargs):
    from concourse.bass_utils import run_bass_kernel_spmd

    nc = _get_program()
    in_maps = _prep_inputs(x, attn_mask, w_qkv, w_proj, b_proj)
    res = run_bass_kernel_spmd(
        nc, in_maps, list(range(NCORES)), trace=trace, **spmd_kwargs
    )
    outs = []
    for b in range(B):
        o = np.asarray(res.results[b]["out"])                       # [128, 8*768]
        outs.append(
            o.reshape(P, LT, D).transpose(1, 0, 2).reshape(L, D)
        )
    return np.stack(outs, axis=0).astype(np.float32), res


def kernel(x, attn_mask, w_qkv, w_proj, b_proj):
    out, _ = run(x, attn_mask, w_qkv, w_proj, b_proj)
    return out
